# revision 1
# baseline (speedup 1.0000x reference)
"""Trainium2 Bass kernel for nn_MultiHeadAttention_42640435315371.

Data-parallel over 8 NeuronCores: each core handles 2048 of the 16384
(n*t) tokens; the four d_model x d_model weights are replicated (shipped
bf16, pre-transposed/permuted on host).

Math notes (matching reference.py exactly):
  - energy_t = Qh_t^T Kh_t / 32 per token (token-local "attention");
    the 1/32 scale and the mask are folded into K as K * mask/32, so a
    masked token yields an all-zero energy matrix -> softmax = uniform
    1/64, identical to softmax of a constant -1e20 row.
  - energies are tiny (|E| < ~1), so exp() needs no max-subtraction.
  - concat order is (d_head, head); Wo's columns are permuted on host so
    the device can emit rows k = h*64 + i.

v2 scheduling: DMA staging merged into few large transfers; DRAM staging
buffers written in source order (sequential writes, strided reads);
engine split: ACT=exp only, DVE=copies/normalize, sync-ring=Q/K/V staging,
SWDGE(gpsimd)=x loads + head staging; deeper pools for cross-megatile
overlap.
"""

import numpy as np

import concourse.bass as bass
import concourse.mybir as mybir
from concourse import bacc
from concourse.tile import TileContext
from concourse.bass_utils import run_bass_kernel_spmd

F32 = mybir.dt.float32
BF16 = mybir.dt.bfloat16

N_CORES = 8
N, T, D, H, DH = 4, 4096, 1024, 16, 64
TOK = (N * T) // N_CORES  # 2048 tokens per core
MT = 512                  # megatile tokens
import os
NMT = int(os.environ.get('K_NMT', TOK // MT))
STAGE = int(os.environ.get('K_STAGE', 99))


def build_nc():
    nc = bacc.Bacc("TRN2", target_bir_lowering=False, debug=False,
                   num_devices=N_CORES)
    xq = nc.declare_dram_parameter("xq", [D, TOK], F32, isOutput=False)
    xk = nc.declare_dram_parameter("xk", [D, TOK], F32, isOutput=False)
    xv = nc.declare_dram_parameter("xv", [D, TOK], F32, isOutput=False)
    wq = nc.declare_dram_parameter("wq", [D, D], BF16, isOutput=False)
    wk = nc.declare_dram_parameter("wk", [D, D], BF16, isOutput=False)
    wv = nc.declare_dram_parameter("wv", [D, D], BF16, isOutput=False)
    wo = nc.declare_dram_parameter("wo", [D, D], BF16, isOutput=False)
    m32 = nc.declare_dram_parameter("m32", [128, TOK // 128], F32, isOutput=False)
    ident = nc.declare_dram_parameter("ident", [128, 128], BF16, isOutput=False)
    out = nc.declare_dram_parameter("out", [D, TOK], F32, isOutput=True)

    from contextlib import ExitStack
    with TileContext(nc) as tc, ExitStack() as ctx:
        const = ctx.enter_context(tc.tile_pool(name="const", bufs=1))
        p_xb = ctx.enter_context(tc.tile_pool(name="xb", bufs=8))
        p_maj = ctx.enter_context(tc.tile_pool(name="maj", bufs=2))
        p_cc = ctx.enter_context(tc.tile_pool(name="cc", bufs=8))
        p_exp = ctx.enter_context(tc.tile_pool(name="expp", bufs=3))
        p_shs = ctx.enter_context(tc.tile_pool(name="shs", bufs=2))
        p_rcp = ctx.enter_context(tc.tile_pool(name="rcp", bufs=2))
        p_outT = ctx.enter_context(tc.tile_pool(name="outT", bufs=2))
        ps_proj = ctx.enter_context(tc.tile_pool(name="psp", bufs=2, space="PSUM"))
        ps_E = ctx.enter_context(tc.tile_pool(name="psE", bufs=3, space="PSUM"))
        ps_2 = ctx.enter_context(tc.tile_pool(name="ps2", bufs=3, space="PSUM"))
        p_stage = ctx.enter_context(tc.tile_pool(name="stage", bufs=2,
                                                 space="DRAM"))

        # ---- static tiles ----
        def load_w(name, dram):
            tiles = []
            for i in range(8):
                t = const.tile([128, D], BF16, tag=f"{name}{i}")
                nc.sync.dma_start(out=t[:], in_=dram[i * 128:(i + 1) * 128, :])
                tiles.append(t)
            return tiles

        wq_sb, wk_sb, wv_sb, wo_sb = (load_w(n, d) for n, d in
                                      (("wq", wq), ("wk", wk), ("wv", wv), ("wo", wo)))
        m32_sb = const.tile([128, TOK // 128], F32, tag="m32")
        nc.sync.dma_start(out=m32_sb[:], in_=m32[:])
        # Packed per-tc4 shuffle tiles (2x ping-pong, zeros static).
        # stqT [32=(b,h), (gf g2 i)]; bdkT [32=(b,h), (gf g2 y j)]
        # block-diagonal in (b,y); bdvT [128=(b,j), 64g*34(b',h | ones)].
        stqT_pp, bdkT_pp, bdvT_pp = [], [], []
        for i in range(2):
            t = const.tile([32, 64 * 64], BF16, tag=f"stqT{i}")
            stqT_pp.append(t)
            t = const.tile([32, 64 * 128], BF16, tag=f"bdkT{i}")
            nc.vector.memset(t[:], 0.0)
            bdkT_pp.append(t)
            t = const.tile([128, 64 * 34], BF16, tag=f"bdvT{i}")
            nc.vector.memset(t[:], 0.0)
            for b in range(2):
                # ones column at (b',h)-col 32+b for row-half b
                nc.vector.memset(
                    t[b * 64:(b + 1) * 64, :].rearrange(
                        "j (g c) -> j g c", c=34)[:, :, 32 + b:33 + b], 1.0)
            bdvT_pp.append(t)

        Copy = mybir.ActivationFunctionType.Copy
        Exp = mybir.ActivationFunctionType.Exp
        Mult = mybir.AluOpType.mult

        for mt in range(NMT):
            t0 = mt * MT
            # ---- load x megatile, cast to bf16 (SWDGE casts in flight) ----
            def load_x(dram, name):
                sbs = []
                for kc in range(8):
                    tb = p_xb.tile([128, MT], BF16, tag=f"x{name}")
                    nc.gpsimd.dma_start(out=tb[:],
                                        in_=dram[kc * 128:(kc + 1) * 128,
                                                 t0:t0 + MT])
                    sbs.append(tb)
                return sbs

            xq_sb = load_x(xq, "q")
            xk_sb = load_x(xk, "k")
            xv_sb = load_x(xv, "v")

            # ---- projections (T-major: out[t_chunk, o]) ----
            qmaj, kmaj, vmaj = [], [], []
            for tc4 in range(4):
                qm = p_maj.tile([128, D], BF16, tag="qmaj")
                km = p_maj.tile([128, D], BF16, tag="kmaj")
                vm = p_maj.tile([128, D], BF16, tag="vmaj")
                for dst, xsb, wsb, is_k in ((qm, xq_sb, wq_sb, False),
                                            (km, xk_sb, wk_sb, True),
                                            (vm, xv_sb, wv_sb, False)):
                    pss = [ps_proj.tile([128, 512], F32, tag="psp",
                                        name=f"psp{mt}_{tc4}_{id(dst)}_{i}")
                           for i in range(2)]
                    for kc in range(8):
                        for oc2 in range(2):
                            nc.tensor.matmul(
                                out=pss[oc2][:],
                                lhsT=xsb[kc][:, tc4 * 128:(tc4 + 1) * 128],
                                rhs=wsb[kc][:, oc2 * 512:(oc2 + 1) * 512],
                                start=(kc == 0), stop=(kc == 7))
                    for oc2 in range(2):
                        dslice = dst[:, oc2 * 512:(oc2 + 1) * 512]
                        if is_k:
                            mcol = mt * 4 + tc4
                            nc.vector.tensor_scalar(
                                out=dslice, in0=pss[oc2][:],
                                scalar1=m32_sb[:, mcol:mcol + 1], scalar2=None,
                                op0=Mult)
                        else:
                            nc.scalar.activation(out=dslice, in_=pss[oc2][:],
                                                 func=Copy)
                qmaj.append(qm)
                kmaj.append(km)
                vmaj.append(vm)

            if STAGE <= 1:
                for oc in range(8):
                    nc.gpsimd.dma_start(out=out[oc * 128:(oc + 1) * 128,
                                                t0:t0 + MT],
                                        in_=qmaj[oc % 4][:, 0:512])
                continue

            # ---- attention ----
            concatT = [p_cc.tile([128, MT], BF16, tag="cc", name=f"cc{mt}_{i}")
                       for i in range(8)]
            rcp64 = p_rcp.tile([64, MT], F32, tag="rcp64")
            sh = p_stage.tile([16, 64, MT], BF16, tag="sh")  # [h][i][t]
            for tc4 in range(4):
                pp = tc4 % 2
                stqT, bdkT, bdvT = stqT_pp[pp], bdkT_pp[pp], bdvT_pp[pp]
                # -- stage Q/K/V through DRAM: contiguous token-major dumps,
                #    strided read-backs build the packed tiles. Host orders
                #    tokens parity-major per 128-block (row b*64+g holds
                #    original token 2g+b), so each b-half is a contiguous
                #    row range. --
                sq = p_stage.tile([128, 1024], BF16, tag="sq")  # [t', (h i)]
                sk = p_stage.tile([128, 1024], BF16, tag="sk")  # [t', (h j)]
                sv = p_stage.tile([128, 1024], BF16, tag="sv")  # [t', (j h)]
                nc.gpsimd.dma_start(out=sq[:], in_=qmaj[tc4][:])
                nc.gpsimd.dma_start(out=sk[:], in_=kmaj[tc4][:])
                nc.gpsimd.dma_start(out=sv[:], in_=vmaj[tc4][:])
                for b in range(2):
                    half = slice(b * 64, (b + 1) * 64)
                    nc.sync.dma_start(
                        out=stqT[b * 16:(b + 1) * 16, :].rearrange(
                            "h (g i) -> h g i", i=64),
                        in_=sq[half].rearrange("g (h i) -> h g i", i=64))
                    nc.sync.dma_start(
                        out=bdkT[b * 16:(b + 1) * 16, :].rearrange(
                            "h (g y j) -> h g y j",
                            y=2, j=64)[:, :, b, :],
                        in_=sk[half].rearrange("g (h j) -> h g j", j=64))
                    nc.sync.dma_start(
                        out=bdvT[b * 64:(b + 1) * 64, :].rearrange(
                            "j (g c) -> j g c", c=34)[:, :, b * 16:(b + 1) * 16],
                        in_=sv[half].rearrange("g (j h) -> j g h", h=16))
                if STAGE <= 2:
                    continue
                # per-tc4 head accumulator [i, h, t_local] bf16
                sh_sb = p_shs.tile([64, 16, 128], BF16, tag="shs")
                for batch in range(8):  # 16 tokens
                    bt = tc4 * 8 + batch
                    psE = ps_E.tile([128, 512], F32, tag="psE")
                    for g8 in range(8):
                        g = batch * 8 + g8      # group in tc4 (2 tokens)
                        nc.tensor.matmul(
                            out=psE[:, g8 * 64:(g8 + 1) * 64],
                            lhsT=bdkT[:, g * 128:(g + 1) * 128],
                            rhs=stqT[:, g * 64:(g + 1) * 64],
                            start=True, stop=True)
                    expE = p_exp.tile([128, 512], BF16, tag="expE")
                    nc.scalar.activation(out=expE[:], in_=psE[:], func=Exp)
                    ps2 = ps_2.tile([64, 272], F32, tag="ps2")
                    for g8 in range(8):
                        g = batch * 8 + g8
                        nc.tensor.matmul(
                            out=ps2[:, g8 * 34:(g8 + 1) * 34],
                            lhsT=expE[:, g8 * 64:(g8 + 1) * 64],
                            rhs=bdvT[:, g * 34:(g + 1) * 34],
                            start=True, stop=True)
                    ps2v = ps2[:].rearrange("p (g c) -> p g c", c=34)
                    nc.vector.reciprocal(
                        rcp64[:, bt * 16:(bt + 1) * 16].rearrange(
                            "p (g b) -> p g b", b=2),
                        ps2v[:, :, 32:34])
                    # head rows into per-tc4 accumulator: free = (h, t16)
                    nc.vector.tensor_copy(
                        sh_sb[:, :, batch * 16:(batch + 1) * 16].rearrange(
                            "p h (g b) -> p g b h", g=8, b=2),
                        ps2v[:, :, 0:32].rearrange("p g (b h) -> p g b h", h=16))
                # one staged write per tc4: [h][i][128t]
                nc.gpsimd.dma_start(
                    out=sh[:, :, tc4 * 128:(tc4 + 1) * 128].rearrange(
                        "h i t -> i h t"),
                    in_=sh_sb[:])
            if STAGE <= 2:
                for oc in range(8):
                    nc.gpsimd.dma_start(out=out[oc * 128:(oc + 1) * 128,
                                                t0:t0 + MT],
                                        in_=bdkT_pp[oc % 2][:, 0:1024])
                continue
            for kc in range(8):
                nc.scalar.dma_start(out=concatT[kc][:],
                                    in_=sh[2 * kc:2 * kc + 2])
            if STAGE <= 3:
                for oc in range(8):
                    nc.sync.dma_start(out=out[oc * 128:(oc + 1) * 128,
                                              t0:t0 + MT],
                                      in_=concatT[oc][:])
                continue

            # ---- normalize + output projection ----
            rcp128 = p_rcp.tile([128, MT], F32, tag="rcp128")
            nc.vector.tensor_copy(rcp128[0:64, :], rcp64[:])
            nc.scalar.dma_start(out=rcp128[64:128, :], in_=rcp64[:])
            ccb = []
            for kc in range(8):
                cb = p_cc.tile([128, MT], BF16, tag="ccb")
                nc.vector.tensor_tensor(out=cb[:], in0=concatT[kc][:],
                                        in1=rcp128[:], op=Mult)
                ccb.append(cb)
            for oc in range(8):
                ps = ps_proj.tile([128, 512], F32, tag="psp")
                for kc in range(8):
                    nc.tensor.matmul(out=ps[:],
                                     lhsT=wo_sb[kc][:, oc * 128:(oc + 1) * 128],
                                     rhs=ccb[kc][:],
                                     start=(kc == 0), stop=(kc == 7))
                ot = p_outT.tile([128, MT], F32, tag="outT")
                nc.vector.tensor_copy(ot[:], ps[:])
                nc.scalar.dma_start(out=out[oc * 128:(oc + 1) * 128,
                                                t0:t0 + MT], in_=ot[:])
    nc.compile()
    return nc


_NC_CACHE = None


def _get_nc():
    global _NC_CACHE
    if _NC_CACHE is None:
        _NC_CACHE = build_nc()
    return _NC_CACHE


def _host_prep(queries, keys, values, mask, Wq, Wk, Wv, Wo):
    """Build the 8 per-core input maps."""
    fq = np.ascontiguousarray(queries.reshape(N * T, D).T)  # [D, 16384]
    fk = np.ascontiguousarray(keys.reshape(N * T, D).T)
    fv = np.ascontiguousarray(values.reshape(N * T, D).T)
    fm = mask.reshape(N * T).astype(np.float32) / 32.0

    import ml_dtypes
    bf = lambda a: np.ascontiguousarray(a).astype(ml_dtypes.bfloat16)
    wq_h = bf(Wq.T)
    wk_h = bf(Wk.T)
    ov = np.arange(D)
    perm_v = (ov % 16) * 64 + (ov // 16)  # device col j*16+h <- orig h*64+j
    wv_h = bf(Wv.T[:, perm_v])
    kpp = np.arange(D)
    perm = (kpp % 64) * 16 + (kpp // 64)  # k''=h*64+i -> source row i*16+h
    wo_h = bf(Wo.T[perm])
    ident = np.eye(128, dtype=np.float32).astype(ml_dtypes.bfloat16)

    # Parity-major token order per 128-block: new position p = b*64 + g
    # holds original token 2g + b (so each pair-half is a contiguous
    # partition range on device).
    p128 = np.arange(128)
    perm128 = 2 * (p128 % 64) + (p128 // 64)
    gperm = np.concatenate([blk * 128 + perm128 for blk in range(TOK // 128)])

    in_maps = []
    for c in range(N_CORES):
        s = slice(c * TOK, (c + 1) * TOK)
        in_maps.append({
            "xq": np.ascontiguousarray(fq[:, s][:, gperm]),
            "xk": np.ascontiguousarray(fk[:, s][:, gperm]),
            "xv": np.ascontiguousarray(fv[:, s][:, gperm]),
            "wq": wq_h, "wk": wk_h, "wv": wv_h, "wo": wo_h,
            "m32": np.ascontiguousarray(
                fm[s][gperm].reshape(TOK // 128, 128).T),
            "ident": ident,
        })
    return in_maps


def kernel(queries, keys, values, mask, Wq, Wk, Wv, Wo, _trace=False, _tmpdir=None):
    queries = np.asarray(queries, dtype=np.float32)
    keys = np.asarray(keys, dtype=np.float32)
    values = np.asarray(values, dtype=np.float32)
    mask = np.asarray(mask)
    in_maps = _host_prep(queries, keys, values, mask,
                         np.asarray(Wq, np.float32), np.asarray(Wk, np.float32),
                         np.asarray(Wv, np.float32), np.asarray(Wo, np.float32))
    nc = _get_nc()
    res = run_bass_kernel_spmd(nc, in_maps, core_ids=list(range(N_CORES)),
                               trace=_trace, tmpdir=_tmpdir)
    outs = []
    for c in range(N_CORES):
        outs.append(np.asarray(res.results[c]["out"]).T)  # [TOK, D]
    full = np.concatenate(outs, axis=0).reshape(N, T, D)
    kernel.last_exec_time_ns = res.exec_time_ns
    return full

